# revision 3
# baseline (speedup 1.0000x reference)
"""GAT-style attention kernel for Trainium2, data-parallel over batch on 8 cores.

Math (same derivation as baseline): rank-1 score structure makes lr_row cancel
in the softmax, so
    out = (M @ (w * xv0)) / (M @ w) + bx,   w[j] = exp(lr_col[j]), xv0 = x @ Wx.T
(no max-subtraction needed: lr_col is O(1), exp cannot overflow).

v2 design vs baseline: the mask transpose moves OFF the DMA fabric (the xbar
transpose generated ~265B packets that poisoned the shared SDMA engines and
held mask loads to ~145GB/s).  Per strip ti:
  - sync HWDGE loads mask strip pairs [128, 2, N] int32 (2MB per dma)
  - DVE casts one strip [128, N] i32->bf16 (2x perf mode)
  - PE transposes the strip: 16 plain matmuls lhsT=mask chunk, rhs=identity
    -> bf16 PSUM tiles [128, 512] (4 chunks each), evacuated to SBUF by
    DVE/ACT alternating (bf16 keeps DVE in 2x mode)
  - PE accumulates pacc[i, 132] over 16 chunks: lhsT=mT chunk, rhs=U chunk
    (U[:,0:128]=w*xv0, U[:,128]=w), interleaved with the next strip's
    transpose matmuls so the PE never waits on evacuation
  - phase B per strip straight from PSUM: DVE reciprocal of the denom col,
    ACT scale-copy, DVE +bx, SWDGE (gpsimd) store
DMA then carries only the compulsory 18MB/core -> memory roofline ~50us.
"""

import os
import sys

import numpy as np

for _p in ("/opt/trn_rl_repo",):
    if _p not in sys.path and os.path.isdir(_p):
        sys.path.append(_p)

import concourse.bacc as bacc
import concourse.bass as bass
import concourse.tile as tile
from concourse import mybir
from concourse.bass_utils import run_bass_kernel_spmd

B, N, DIN, DOUT, DA = 8, 2048, 128, 128, 2
NEG_SLOPE = 0.2
P = 128
UC = 132  # U chunk width: 128 numerator cols + 1 denom col + 3 pad

F32 = mybir.dt.float32
BF16 = mybir.dt.bfloat16
I32 = mybir.dt.int32


def build(n=N, mask_bufs=3, cast_bufs=10, mt_bufs=5, pair=2, phaseb_skew=1,
          m_skew=3, cast="swdge", gsz=8,
          evac_pat=("vector", "scalar"), tmode="xpose_bf16"):
    """Build the single-core program (all 8 cores run it SPMD).

    cast:  "swdge" — gpsimd cast-during-DMA loads (i32->bf16 inline, no DVE cast)
           "dve"   — sync HWDGE pair loads + DVE tensor_copy cast
    gsz:   transposes batched per PSUM tile (8 -> one full 2KB bank in bf16)
    m_skew: M-phase of strip k runs in iteration k+m_skew (pipeline depth)
    """
    nt = n // P
    gsz = min(gsz, nt)
    assert nt % 4 == 0 and nt % pair == 0 and nt % gsz == 0
    ngm = nt // gsz   # transpose groups per strip (main loop)
    ng4 = nt // 4     # prologue groups of 4
    nc = bacc.Bacc(
        "TRN2",
        target_bir_lowering=False,
        debug=False,
        enable_asserts=False,
        num_devices=1,
    )
    xbf_d = nc.dram_tensor("xbf", [P, nt, DIN], BF16, kind="ExternalInput").ap()
    m_d = nc.dram_tensor("mask", [n, n], I32, kind="ExternalInput").ap()
    wxT_d = nc.dram_tensor("wxT", [DIN, DOUT], BF16, kind="ExternalInput").ap()
    wcT_d = nc.dram_tensor("wcT", [DIN, DA], BF16, kind="ExternalInput").ap()
    a2_d = nc.dram_tensor("a2", [P, DA], F32, kind="ExternalInput").ap()
    bx_d = nc.dram_tensor("bx", [P, DOUT], F32, kind="ExternalInput").ap()
    ident_d = nc.dram_tensor("ident", [P, P], BF16, kind="ExternalInput").ap()
    out_d = nc.dram_tensor("out", [n, DOUT], F32, kind="ExternalOutput").ap()

    m_v = m_d.rearrange("(q p) j -> p q j", p=P)  # [P, nt, n] strip view

    from contextlib import ExitStack

    with tile.TileContext(nc) as tc, ExitStack() as ctx:
        consts = ctx.enter_context(tc.tile_pool(name="consts", bufs=1))
        small = ctx.enter_context(tc.tile_pool(name="small", bufs=2))
        if cast == "dve":
            mpool = ctx.enter_context(tc.tile_pool(name="mpool", bufs=mask_bufs))
        cpool = ctx.enter_context(tc.tile_pool(name="cpool", bufs=cast_bufs))
        tpool = ctx.enter_context(tc.tile_pool(name="tpool", bufs=mt_bufs))
        opool = ctx.enter_context(tc.tile_pool(name="opool", bufs=4))
        ps_xm = ctx.enter_context(tc.tile_pool(name="ps_xm", bufs=3, space="PSUM"))
        ps_acc = ctx.enter_context(tc.tile_pool(name="ps_acc", bufs=3, space="PSUM"))
        ps_pxv = ctx.enter_context(tc.tile_pool(name="ps_pxv", bufs=1, space="PSUM"))

        ev = {"vector": nc.vector.tensor_copy, "scalar": nc.scalar.copy}
        psx_dt = BF16 if tmode == "xpose_bf16" else F32

        def pe_transpose(out_ps, in_sb):
            if tmode == "xpose_bf16":
                nc.tensor.transpose(out_ps, in_sb, identB[:])
            else:
                nc.tensor.matmul(out_ps, in_sb, identB[:], start=True, stop=True)

        # ---- consts split across both HWDGE queues to land x early ----
        xbf = consts.tile([P, nt, DIN], BF16)
        nxc = max(1, nt // 4)
        for c in range(nxc):
            lo = c * (nt // nxc)
            hi = (c + 1) * (nt // nxc)
            q = nc.sync if (cast == "swdge" and c % 2 == 0) else nc.scalar
            q.dma_start(xbf[:, lo:hi], xbf_d[:, lo:hi])
        identB = consts.tile([P, P], BF16)
        (nc.sync if cast == "swdge" else nc.scalar).dma_start(identB[:], ident_d)
        wxT = consts.tile([DIN, DOUT], BF16)
        nc.scalar.dma_start(wxT[:], wxT_d)
        wcT = consts.tile([DIN, DA], BF16)
        nc.scalar.dma_start(wcT[:], wcT_d)
        a2b = consts.tile([P, DA], F32)
        nc.scalar.dma_start(a2b[:], a2_d)
        bxb = consts.tile([P, DOUT], F32)
        nc.scalar.dma_start(bxb[:], bx_d)

        # ---- x -> xT via PE transposes, packed gsz/psum tile ----
        xT = consts.tile([P, n], BF16)
        for g in range(ngm):
            psx = ps_xm.tile([P, gsz * P], psx_dt, name="psxm", tag="psxm")
            for q in range(gsz):
                t = gsz * g + q
                pe_transpose(psx[:, q * P : (q + 1) * P], xbf[:, t])
            ev[evac_pat[g % len(evac_pat)]](
                xT[:, gsz * g * P : (gsz * g + gsz) * P], psx[:]
            )

        # ---- col projection (N=2) first so the w chain runs early ----
        pcol = ps_pxv.tile([P, nt * DA], F32, tag="pcol")
        for t in range(nt):
            nc.tensor.matmul(
                pcol[:, t * DA : (t + 1) * DA],
                xT[:, t * P : (t + 1) * P],
                wcT[:],
                start=True,
                stop=True,
            )
        colv = small.tile([P, nt, DA], F32)
        nc.vector.tensor_copy(colv[:], pcol[:].rearrange("p (t a) -> p t a", a=DA))
        c02 = small.tile([P, nt, DA], F32)
        nc.vector.tensor_scalar_mul(c02[:], colv[:], NEG_SLOPE)
        clr = small.tile([P, nt, DA], F32)
        nc.vector.tensor_max(clr[:], colv[:], c02[:])
        lr0 = small.tile([P, nt], F32)
        nc.vector.tensor_scalar(
            lr0[:], clr[:, :, 0], a2b[:, 0:1], None, mybir.AluOpType.mult
        )
        lr1 = small.tile([P, nt], F32)
        nc.vector.tensor_scalar(
            lr1[:], clr[:, :, 1], a2b[:, 1:2], None, mybir.AluOpType.mult
        )
        lrc = small.tile([P, nt], F32)
        nc.vector.tensor_add(lrc[:], lr0[:], lr1[:])
        w_all = consts.tile([P, nt], F32)
        nc.scalar.activation(w_all[:], lrc[:], mybir.ActivationFunctionType.Exp)

        # ---- U chunks: U[:,t,0:128]=w*(xv0+bx), U[:,t,128]=w  (bias folded) ----
        U = consts.tile([P, nt, UC], BF16)
        nc.vector.memset(U[:], 0)
        xvb = consts.tile([P, nt, DOUT], F32)
        for g in range(ng4):
            psv = ps_pxv.tile([P, 4 * P], F32, tag="psv")
            for q in range(4):
                t = 4 * g + q
                nc.tensor.matmul(
                    psv[:, q * P : (q + 1) * P],
                    xT[:, t * P : (t + 1) * P],
                    wxT[:],
                    start=True,
                    stop=True,
                )
            for q in range(4):
                t = 4 * g + q
                nc.vector.tensor_add(xvb[:, t], psv[:, q * P : (q + 1) * P], bxb[:])
            for q in range(4):
                t = 4 * g + q
                if q % 2 == 0:
                    nc.scalar.activation(
                        U[:, t, 0:DOUT],
                        xvb[:, t],
                        mybir.ActivationFunctionType.Copy,
                        scale=w_all[:, t : t + 1],
                    )
                else:
                    nc.vector.tensor_scalar(
                        U[:, t, 0:DOUT],
                        xvb[:, t],
                        w_all[:, t : t + 1],
                        None,
                        mybir.AluOpType.mult,
                    )
        nc.vector.tensor_copy(U[:, :, DOUT], w_all[:])

        # ---- main loop: software-pipelined; iter k runs T(k) + M(k-m_skew) ----
        mis = {}
        mbfs = {}
        mTs = {}
        paccs = {}
        store_q = nc.sync if cast == "swdge" else nc.gpsimd

        def phase_b(ti):
            pacc = paccs.pop(ti)
            rec = small.tile([P, 1], F32, tag="rec")
            nc.vector.reciprocal(rec[:], pacc[:, DOUT : DOUT + 1])
            o1 = opool.tile([P, DOUT], F32, tag="o1")
            nc.vector.tensor_scalar(
                o1[:], pacc[:, 0:DOUT], rec[:], None, mybir.AluOpType.mult
            )
            store_q.dma_start(out_d[ti * P : (ti + 1) * P, :], o1[:])

        for k in range(nt + m_skew):
            if k < nt:
                if cast == "swdge":
                    mbf = cpool.tile([P, n], BF16, tag="mbf")
                    nc.gpsimd.dma_start(mbf[:], m_v[:, k])
                    mbfs[k] = mbf
                else:
                    if k % pair == 0:
                        mi = mpool.tile([P, pair, n], I32, tag="mi")
                        nc.sync.dma_start(mi[:], m_v[:, k : k + pair])
                        for a in range(pair):
                            mis[k + a] = (mi, a)
                    mi, a = mis.pop(k)
                    mbf = cpool.tile([P, n], BF16, tag="mbf")
                    nc.vector.tensor_copy(mbf[:], mi[:, a])
                    mbfs[k] = mbf
                mTs[k] = tpool.tile([P, n], BF16, name="mT", tag="mT")
            if k >= m_skew + phaseb_skew:
                phase_b(k - m_skew - phaseb_skew)
            if k >= m_skew:
                paccs[k - m_skew] = ps_acc.tile([P, UC], F32, name="pacc", tag="pacc")
            for g in range(ngm):
                if k < nt:
                    mbf = mbfs[k]
                    psx = ps_xm.tile([P, gsz * P], psx_dt, name="psxm", tag="psxm")
                    for q in range(gsz):
                        t = gsz * g + q
                        pe_transpose(
                            psx[:, q * P : (q + 1) * P],
                            mbf[:, t * P : (t + 1) * P],
                        )
                if k >= m_skew:
                    mT = mTs[k - m_skew]
                    for q in range(gsz):
                        tj = gsz * g + q
                        nc.tensor.matmul(
                            paccs[k - m_skew][:],
                            mT[:, tj * P : (tj + 1) * P],
                            U[:, tj],
                            start=(tj == 0),
                            stop=(tj == nt - 1),
                        )
                if k < nt:
                    ev[evac_pat[g % len(evac_pat)]](
                        mTs[k][:, gsz * g * P : (gsz * g + gsz) * P], psx[:]
                    )
            if k < nt:
                mbfs.pop(k)
            if k >= m_skew:
                mTs.pop(k - m_skew)
        for ti in range(nt - phaseb_skew, nt):
            phase_b(ti)

    nc.compile()
    return nc


def host_inputs(x, mask, Wc, Wcat, Wx, bx, b):
    """Per-core input map for batch b (weights replicated, host-prepped)."""
    import ml_dtypes

    nt = N // P
    xb = np.asarray(x[b], dtype=np.float32)
    xprep = xb.reshape(nt, P, DIN).transpose(1, 0, 2)  # [P, nt, DIN]
    return {
        "xbf": np.ascontiguousarray(xprep, dtype=ml_dtypes.bfloat16),
        "mask": np.ascontiguousarray(mask[b], dtype=np.int32),
        "wxT": np.ascontiguousarray(Wx.T, dtype=ml_dtypes.bfloat16),
        "wcT": np.ascontiguousarray(Wc.T, dtype=ml_dtypes.bfloat16),
        "a2": np.ascontiguousarray(
            np.broadcast_to(Wcat[DA:].reshape(1, DA), (P, DA)), dtype=np.float32
        ),
        "bx": np.ascontiguousarray(
            np.broadcast_to(bx.reshape(1, DOUT), (P, DOUT)), dtype=np.float32
        ),
        "ident": np.eye(P, dtype=ml_dtypes.bfloat16),
    }


_cached = {}


def _get_nc():
    if "nc" not in _cached:
        _cached["nc"] = build()
    return _cached["nc"]


def _install_ntff_shim():
    """The agent image's antenv lacks axon_hooks; synthesize it so
    run_bass_kernel_spmd(trace=True) can reach the .so's NTFF profiler."""
    import types

    try:
        import antenv.axon_hooks  # noqa: F401

        return True
    except ImportError:
        pass
    try:
        import antenv
        from trn_agent_boot.trn_boot import _ntff_profile_via_ctypes

        hook = _ntff_profile_via_ctypes("/opt/axon/libaxon_pjrt.so")
        mod = types.ModuleType("antenv.axon_hooks")
        _state = {"hook": hook}
        mod.set_axon_ntff_profile_hook = lambda h: _state.__setitem__("hook", h)
        mod.get_axon_ntff_profile_hook = lambda: _state["hook"]
        sys.modules["antenv.axon_hooks"] = mod
        antenv.axon_hooks = mod
        return hook is not None
    except Exception as e:
        print(f"ntff shim failed: {e}", file=sys.stderr)
        return False


def kernel(x, mask, Wr, Wc, Wcat, Wx, bx, _trace=False, **_unused):
    x = np.asarray(x)
    mask = np.asarray(mask)
    Wc = np.asarray(Wc)
    Wcat = np.asarray(Wcat)
    Wx = np.asarray(Wx)
    bx = np.asarray(bx)
    nc = _get_nc()
    if _trace:
        _trace = _install_ntff_shim()
    in_maps = [host_inputs(x, mask, Wc, Wcat, Wx, bx, b) for b in range(B)]
    res = run_bass_kernel_spmd(nc, in_maps, core_ids=list(range(B)), trace=_trace)
    out = np.stack([res.results[c]["out"] for c in range(B)]).astype(np.float32)
    if _trace:
        kernel.last_results = res
    return out


# revision 4
# speedup vs baseline: 1.0408x; 1.0408x over previous
"""GAT-style attention kernel for Trainium2, data-parallel over batch on 8 cores.

Math: the rank-1 score structure e[i,j] = lr_row[i] + lr_col[j] makes lr_row
cancel in the softmax, so
    out = (M @ (w * xvb)) / (M @ w),
    w[j] = exp(lr_col[j]),  xvb = x @ Wx.T + bx  (bias folded into U;
    attention rows sum to 1).  No max-subtraction: lr_col is O(1).

Design (70.9us HW, vs 159.7us baseline):
  - mask strips [128, N] load via gpsimd SWDGE cast-DMA (i32->bf16 inline,
    ~330 GB/s HBM read; no DVE cast, half the SBUF write traffic).  The
    baseline's xbar DMA transpose emitted ~265B packets that throttled the
    shared SDMA engines to 145 GB/s - all transposes now run on the PE.
  - PE transposes each strip: 16 transpose-mode matmuls vs identity into
    bf16 PSUM tiles (8 chunks / 2KB bank), evacuated by DVE (2x perf mode)
    and ACT alternating into SBUF.
  - PE accumulates pacc[i, 132] over 16 chunks: lhsT = transposed mask
    chunk, rhs = U chunk (U[:,0:128] = w*xvb, U[:,128] = w), software-
    pipelined with m_skew=3 (M-phase of strip k in iteration k+3).
  - phase B per strip from PSUM: DVE reciprocal of the denom column +
    per-partition multiply; stores on the sync HWDGE queue.
"""

import os
import sys

import numpy as np

for _p in ("/opt/trn_rl_repo",):
    if _p not in sys.path and os.path.isdir(_p):
        sys.path.append(_p)

import concourse.bacc as bacc
import concourse.bass as bass
import concourse.tile as tile
from concourse import mybir
from concourse.bass_utils import run_bass_kernel_spmd

B, N, DIN, DOUT, DA = 8, 2048, 128, 128, 2
NEG_SLOPE = 0.2
P = 128
UC = 132  # U chunk width: 128 numerator cols + 1 denom col + 3 pad

F32 = mybir.dt.float32
BF16 = mybir.dt.bfloat16
I32 = mybir.dt.int32


def build(n=N, mask_bufs=3, cast_bufs=10, mt_bufs=5, pair=2, phaseb_skew=1,
          m_skew=3, cast="swdge", gsz=8,
          evac_pat=("vector", "scalar"), tmode="xpose_bf16"):
    """Build the single-core program (all 8 cores run it SPMD).

    cast:  "swdge" — gpsimd cast-during-DMA loads (i32->bf16 inline, no DVE cast)
           "dve"   — sync HWDGE pair loads + DVE tensor_copy cast
    gsz:   transposes batched per PSUM tile (8 -> one full 2KB bank in bf16)
    m_skew: M-phase of strip k runs in iteration k+m_skew (pipeline depth)
    """
    nt = n // P
    gsz = min(gsz, nt)
    assert nt % 4 == 0 and nt % pair == 0 and nt % gsz == 0
    ngm = nt // gsz   # transpose groups per strip (main loop)
    ng4 = nt // 4     # prologue groups of 4
    nc = bacc.Bacc(
        "TRN2",
        target_bir_lowering=False,
        debug=False,
        enable_asserts=False,
        num_devices=1,
    )
    xbf_d = nc.dram_tensor("xbf", [P, nt, DIN], BF16, kind="ExternalInput").ap()
    m_d = nc.dram_tensor("mask", [n, n], I32, kind="ExternalInput").ap()
    wxT_d = nc.dram_tensor("wxT", [DIN, DOUT], BF16, kind="ExternalInput").ap()
    wcT_d = nc.dram_tensor("wcT", [DIN, DA], BF16, kind="ExternalInput").ap()
    a2_d = nc.dram_tensor("a2", [P, DA], F32, kind="ExternalInput").ap()
    bx_d = nc.dram_tensor("bx", [P, DOUT], F32, kind="ExternalInput").ap()
    ident_d = nc.dram_tensor("ident", [P, P], BF16, kind="ExternalInput").ap()
    out_d = nc.dram_tensor("out", [n, DOUT], F32, kind="ExternalOutput").ap()

    m_v = m_d.rearrange("(q p) j -> p q j", p=P)  # [P, nt, n] strip view

    from contextlib import ExitStack

    with tile.TileContext(nc) as tc, ExitStack() as ctx:
        consts = ctx.enter_context(tc.tile_pool(name="consts", bufs=1))
        small = ctx.enter_context(tc.tile_pool(name="small", bufs=2))
        if cast == "dve":
            mpool = ctx.enter_context(tc.tile_pool(name="mpool", bufs=mask_bufs))
        cpool = ctx.enter_context(tc.tile_pool(name="cpool", bufs=cast_bufs))
        tpool = ctx.enter_context(tc.tile_pool(name="tpool", bufs=mt_bufs))
        opool = ctx.enter_context(tc.tile_pool(name="opool", bufs=4))
        ps_xm = ctx.enter_context(tc.tile_pool(name="ps_xm", bufs=3, space="PSUM"))
        ps_acc = ctx.enter_context(tc.tile_pool(name="ps_acc", bufs=3, space="PSUM"))
        ps_pxv = ctx.enter_context(tc.tile_pool(name="ps_pxv", bufs=1, space="PSUM"))

        ev = {"vector": nc.vector.tensor_copy, "scalar": nc.scalar.copy}
        psx_dt = BF16 if tmode == "xpose_bf16" else F32

        def pe_transpose(out_ps, in_sb):
            if tmode == "xpose_bf16":
                nc.tensor.transpose(out_ps, in_sb, identB[:])
            else:
                nc.tensor.matmul(out_ps, in_sb, identB[:], start=True, stop=True)

        # ---- consts split across both HWDGE queues to land x early ----
        xbf = consts.tile([P, nt, DIN], BF16)
        nxc = max(1, nt // 4)
        for c in range(nxc):
            lo = c * (nt // nxc)
            hi = (c + 1) * (nt // nxc)
            q = nc.sync if (cast == "swdge" and c % 2 == 0) else nc.scalar
            q.dma_start(xbf[:, lo:hi], xbf_d[:, lo:hi])
        identB = consts.tile([P, P], BF16)
        (nc.sync if cast == "swdge" else nc.scalar).dma_start(identB[:], ident_d)
        wxT = consts.tile([DIN, DOUT], BF16)
        nc.scalar.dma_start(wxT[:], wxT_d)
        wcT = consts.tile([DIN, DA], BF16)
        nc.scalar.dma_start(wcT[:], wcT_d)
        a2b = consts.tile([P, DA], F32)
        nc.scalar.dma_start(a2b[:], a2_d)
        bxb = consts.tile([P, DOUT], F32)
        nc.scalar.dma_start(bxb[:], bx_d)

        # ---- x -> xT via PE transposes, packed gsz/psum tile ----
        xT = consts.tile([P, n], BF16)
        for g in range(ngm):
            psx = ps_xm.tile([P, gsz * P], psx_dt, name="psxm", tag="psxm")
            for q in range(gsz):
                t = gsz * g + q
                pe_transpose(psx[:, q * P : (q + 1) * P], xbf[:, t])
            ev[evac_pat[g % len(evac_pat)]](
                xT[:, gsz * g * P : (gsz * g + gsz) * P], psx[:]
            )

        # ---- col projection (N=2) first so the w chain runs early ----
        pcol = ps_pxv.tile([P, nt * DA], F32, tag="pcol")
        for t in range(nt):
            nc.tensor.matmul(
                pcol[:, t * DA : (t + 1) * DA],
                xT[:, t * P : (t + 1) * P],
                wcT[:],
                start=True,
                stop=True,
            )
        colv = small.tile([P, nt, DA], F32)
        nc.vector.tensor_copy(colv[:], pcol[:].rearrange("p (t a) -> p t a", a=DA))
        c02 = small.tile([P, nt, DA], F32)
        nc.vector.tensor_scalar_mul(c02[:], colv[:], NEG_SLOPE)
        clr = small.tile([P, nt, DA], F32)
        nc.vector.tensor_max(clr[:], colv[:], c02[:])
        lr0 = small.tile([P, nt], F32)
        nc.vector.tensor_scalar(
            lr0[:], clr[:, :, 0], a2b[:, 0:1], None, mybir.AluOpType.mult
        )
        lr1 = small.tile([P, nt], F32)
        nc.vector.tensor_scalar(
            lr1[:], clr[:, :, 1], a2b[:, 1:2], None, mybir.AluOpType.mult
        )
        lrc = small.tile([P, nt], F32)
        nc.vector.tensor_add(lrc[:], lr0[:], lr1[:])
        w_all = consts.tile([P, nt], F32)
        nc.scalar.activation(w_all[:], lrc[:], mybir.ActivationFunctionType.Exp)

        # ---- U chunks: U[:,t,0:128]=w*(xv0+bx), U[:,t,128]=w  (bias folded) ----
        U = consts.tile([P, nt, UC], BF16)
        nc.vector.memset(U[:], 0)
        xvb = consts.tile([P, nt, DOUT], F32)
        for g in range(ng4):
            psv = ps_pxv.tile([P, 4 * P], F32, tag="psv")
            for q in range(4):
                t = 4 * g + q
                nc.tensor.matmul(
                    psv[:, q * P : (q + 1) * P],
                    xT[:, t * P : (t + 1) * P],
                    wxT[:],
                    start=True,
                    stop=True,
                )
            for q in range(4):
                t = 4 * g + q
                nc.vector.tensor_add(xvb[:, t], psv[:, q * P : (q + 1) * P], bxb[:])
            for q in range(4):
                t = 4 * g + q
                if q % 2 == 0:
                    nc.scalar.activation(
                        U[:, t, 0:DOUT],
                        xvb[:, t],
                        mybir.ActivationFunctionType.Copy,
                        scale=w_all[:, t : t + 1],
                    )
                else:
                    nc.vector.tensor_scalar(
                        U[:, t, 0:DOUT],
                        xvb[:, t],
                        w_all[:, t : t + 1],
                        None,
                        mybir.AluOpType.mult,
                    )
        nc.vector.tensor_copy(U[:, :, DOUT], w_all[:])

        # ---- main loop: software-pipelined; iter k runs T(k) + M(k-m_skew) ----
        mis = {}
        mbfs = {}
        mTs = {}
        paccs = {}
        store_q = nc.sync if cast == "swdge" else nc.gpsimd

        def phase_b(ti):
            pacc = paccs.pop(ti)
            rec = small.tile([P, 1], F32, tag="rec")
            nc.vector.reciprocal(rec[:], pacc[:, DOUT : DOUT + 1])
            o1 = opool.tile([P, DOUT], F32, tag="o1")
            nc.vector.tensor_scalar(
                o1[:], pacc[:, 0:DOUT], rec[:], None, mybir.AluOpType.mult
            )
            store_q.dma_start(out_d[ti * P : (ti + 1) * P, :], o1[:])

        for k in range(nt + m_skew):
            if k < nt:
                if cast == "swdge":
                    mbf = cpool.tile([P, n], BF16, tag="mbf")
                    nc.gpsimd.dma_start(mbf[:], m_v[:, k])
                    mbfs[k] = mbf
                else:
                    if k % pair == 0:
                        mi = mpool.tile([P, pair, n], I32, tag="mi")
                        nc.sync.dma_start(mi[:], m_v[:, k : k + pair])
                        for a in range(pair):
                            mis[k + a] = (mi, a)
                    mi, a = mis.pop(k)
                    mbf = cpool.tile([P, n], BF16, tag="mbf")
                    nc.vector.tensor_copy(mbf[:], mi[:, a])
                    mbfs[k] = mbf
                mTs[k] = tpool.tile([P, n], BF16, name="mT", tag="mT")
            if k >= m_skew + phaseb_skew:
                phase_b(k - m_skew - phaseb_skew)
            if k >= m_skew:
                paccs[k - m_skew] = ps_acc.tile([P, UC], F32, name="pacc", tag="pacc")
            for g in range(ngm):
                if k < nt:
                    mbf = mbfs[k]
                    psx = ps_xm.tile([P, gsz * P], psx_dt, name="psxm", tag="psxm")
                    for q in range(gsz):
                        t = gsz * g + q
                        pe_transpose(
                            psx[:, q * P : (q + 1) * P],
                            mbf[:, t * P : (t + 1) * P],
                        )
                if k >= m_skew:
                    mT = mTs[k - m_skew]
                    for q in range(gsz):
                        tj = gsz * g + q
                        nc.tensor.matmul(
                            paccs[k - m_skew][:],
                            mT[:, tj * P : (tj + 1) * P],
                            U[:, tj],
                            start=(tj == 0),
                            stop=(tj == nt - 1),
                        )
                if k < nt:
                    ev[evac_pat[g % len(evac_pat)]](
                        mTs[k][:, gsz * g * P : (gsz * g + gsz) * P], psx[:]
                    )
            if k < nt:
                mbfs.pop(k)
            if k >= m_skew:
                mTs.pop(k - m_skew)
        for ti in range(nt - phaseb_skew, nt):
            phase_b(ti)

    nc.compile()
    return nc


def host_inputs(x, mask, Wc, Wcat, Wx, bx, b):
    """Per-core input map for batch b (weights replicated, host-prepped)."""
    import ml_dtypes

    nt = N // P
    xb = np.asarray(x[b], dtype=np.float32)
    xprep = xb.reshape(nt, P, DIN).transpose(1, 0, 2)  # [P, nt, DIN]
    return {
        "xbf": np.ascontiguousarray(xprep, dtype=ml_dtypes.bfloat16),
        "mask": np.ascontiguousarray(mask[b], dtype=np.int32),
        "wxT": np.ascontiguousarray(Wx.T, dtype=ml_dtypes.bfloat16),
        "wcT": np.ascontiguousarray(Wc.T, dtype=ml_dtypes.bfloat16),
        "a2": np.ascontiguousarray(
            np.broadcast_to(Wcat[DA:].reshape(1, DA), (P, DA)), dtype=np.float32
        ),
        "bx": np.ascontiguousarray(
            np.broadcast_to(bx.reshape(1, DOUT), (P, DOUT)), dtype=np.float32
        ),
        "ident": np.eye(P, dtype=ml_dtypes.bfloat16),
    }


_cached = {}


def _get_nc():
    if "nc" not in _cached:
        _cached["nc"] = build()
    return _cached["nc"]


def _install_ntff_shim():
    """The agent image's antenv lacks axon_hooks; synthesize it so
    run_bass_kernel_spmd(trace=True) can reach the .so's NTFF profiler."""
    import types

    try:
        import antenv.axon_hooks  # noqa: F401

        return True
    except ImportError:
        pass
    try:
        import antenv
        from trn_agent_boot.trn_boot import _ntff_profile_via_ctypes

        hook = _ntff_profile_via_ctypes("/opt/axon/libaxon_pjrt.so")
        mod = types.ModuleType("antenv.axon_hooks")
        _state = {"hook": hook}
        mod.set_axon_ntff_profile_hook = lambda h: _state.__setitem__("hook", h)
        mod.get_axon_ntff_profile_hook = lambda: _state["hook"]
        sys.modules["antenv.axon_hooks"] = mod
        antenv.axon_hooks = mod
        return hook is not None
    except Exception as e:
        print(f"ntff shim failed: {e}", file=sys.stderr)
        return False


def kernel(x, mask, Wr, Wc, Wcat, Wx, bx, _trace=False, **_unused):
    x = np.asarray(x)
    mask = np.asarray(mask)
    Wc = np.asarray(Wc)
    Wcat = np.asarray(Wcat)
    Wx = np.asarray(Wx)
    bx = np.asarray(bx)
    nc = _get_nc()
    if _trace:
        _trace = _install_ntff_shim()
    in_maps = [host_inputs(x, mask, Wc, Wcat, Wx, bx, b) for b in range(B)]
    res = run_bass_kernel_spmd(nc, in_maps, core_ids=list(range(B)), trace=_trace)
    out = np.stack([res.results[c]["out"] for c in range(B)]).astype(np.float32)
    if _trace:
        kernel.last_results = res
    return out
